# revision 38
# baseline (speedup 1.0000x reference)
"""Trainium2 Bass kernel for nn_DepthMemoryCache.

Reference computation (D=8, B=4, S=4096, C=1024, G=64):
    u     = einsum('bsc,gc->bsg', x[-1], W_u)
    keys  = einsum('dbc,gc->dbg', x.mean(2), W_u)
    gates = softmax(einsum('bsg,dbg->bsd', u, keys), axis=-1)
    out   = einsum('dbsc,bsd->bsc', x, gates)

Strategy: shard the sequence axis over 8 cores (core i gets
x[:, :, i*512:(i+1)*512, :]). Per core:

  A) one streaming pass over the 64MB shard computes per-(d,b) column
     sums on PE via float32r indicator matmuls straight off the fp32
     slabs (1 cyc/col, no casts, tf32-ish precision only affects the
     gate path). Slabs for d in {0,1,2} are additionally cast to a
     resident 12MB bf16 SBUF cache so phase B does not re-read them.
     No u/logit work happens in phase A (PE stays far under the DMA
     floor), fixing the DMA starvation the old schedule had.
  B) an 8KB AllReduce completes the keys; q = W_u^T @ keys folds the
     projection into a [C, B*D] tensor so per-block logits need only
     x7^T (PE transposes, idle engine in phase B) and one small bf16
     matmul: logits[s,d] = sum_c x7T[c,s] * q[c,d]. Softmax on ACT.
     The depth-weighted sum streams d in {7,3,4,5,6} from HBM (fp32,
     exact) and reads d in {0,1,2} from the bf16 cache; the 8-term FMA
     chain is split 6 ops on DVE + 3 on GpSimd so DMA stays the
     bottleneck. Output writes ride the ACT HWDGE queue.

HBM traffic per core: 64 (A) + 40 (B) + 8 (write) = 112MB, vs 136MB
for the two-full-pass schedule. All SBUF pools are allocated up front
(disjoint bytes), so phase-B prefetch reads continue through the
collective instead of waiting for phase-A buffer releases.
"""
import sys

sys.path.insert(0, "/opt/trn_rl_repo")

from contextlib import ExitStack

import numpy as np
from concourse import bacc, bass, mybir, tile, masks
from concourse import bass_utils

F32 = mybir.dt.float32
F32R = mybir.dt.float32r
BF16 = mybir.dt.bfloat16

D, B, S, C, G = 8, 4, 4096, 1024, 64
N_CORES = 8
P = 128                 # partition count / block rows
NKC = C // P            # 8 column chunks of 128
CACHED_D = (0, 1, 2, 3)  # depths kept resident in SBUF as bf16
STREAM_D = (7, 4, 5, 6)  # depths re-streamed in phase B (7 first)


def build_body(tc, x, w, y, s_sh):
    """Emit the kernel IR. x:[D,B,s_sh,C], w:[G,C], y:[B,s_sh,C] dram APs."""
    nc = tc.nc
    nj = s_sh // P      # 128-row blocks per (d, b)
    mul, add = mybir.AluOpType.mult, mybir.AluOpType.add
    DB = D * B
    es = ExitStack()

    # ---- all SBUF pools up front: disjoint bytes, no phase-B stall ----
    singles = es.enter_context(tc.tile_pool(name="singles", bufs=1))
    bstream = es.enter_context(tc.tile_pool(name="bstream", bufs=2))
    ybfp = es.enter_context(tc.tile_pool(name="ybfp", bufs=2))
    accp = es.enter_context(tc.tile_pool(name="accp", bufs=2))
    dgp = es.enter_context(tc.tile_pool(name="dgp", bufs=2))
    xtp = es.enter_context(tc.tile_pool(name="xtp", bufs=2))
    gsm = es.enter_context(tc.tile_pool(name="gsm", bufs=3))
    dram = es.enter_context(tc.tile_pool(name="dram", bufs=1, space="DRAM"))

    ident = singles.tile([P, P], F32)
    masks.make_identity(nc, ident[:])
    # f32r copy of the identity: DVE copy output dtype f32r counts as
    # "rounded to FP32r" for the BIR verifier (values 0/1 are exact)
    ident_r = singles.tile([P, P], F32R)
    nc.vector.tensor_copy(ident_r[:], ident[:])
    # indicator stationaries: ind[:, r, m] = (m == r) / S — f32r matmul
    # column-sums a fp32 slab into psum row r at 1 cyc/col.
    ind_f = singles.tile([P, DB, DB], F32)
    nc.vector.memset(ind_f[:], 0.0)
    for r in range(DB):
        nc.vector.memset(ind_f[:, r, r:r + 1], 1.0 / (N_CORES * s_sh))
    w_sb = singles.tile([G, C], F32)
    nc.sync.dma_start(w_sb[:], w[:])
    cache_bf = singles.tile([P, len(CACHED_D), B, nj, C], BF16)
    sums_sb = singles.tile([DB, C], F32)
    sumk_sb = singles.tile([G, B * D], F32)
    meanT_sb = singles.tile([P, NKC * DB], F32)
    wT_sb = singles.tile([P, NKC, G], F32)
    keysT_sb = singles.tile([G, B, D], F32)
    q_bf = singles.tile([P, NKC, B, D], BF16)

    # tiny warm-up AllReduce: absorbs collective-comm setup under phase A
    ccw_in = dram.tile([1, 16], F32)
    ccw_out = dram.tile([1, 16], F32)
    cc_in = dram.tile([G, B * D], F32)
    cc_out = dram.tile([G, B * D], F32)
    warm_sb = singles.tile([1, 16], F32)
    nc.vector.memset(warm_sb[:], 0.0)
    nc.gpsimd.dma_start(ccw_in[:], warm_sb[:])
    nc.gpsimd.collective_compute(
        "AllReduce", add, replica_groups=[list(range(N_CORES))],
        ins=[ccw_in.opt()], outs=[ccw_out.opt()],
    )

    # ---------------- Phase A: partial sums over s (scaled by 1/S) ----------
    # stream pool scoped to phase A: its bytes are re-used by bstream2
    # below for extra phase-B prefetch depth across the collective
    with tc.tile_pool(name="stream", bufs=3) as stream, \
         tc.tile_pool(name="psumA", bufs=1, space="PSUM") as psA, \
         tc.tile_pool(name="psumT", bufs=1, space="PSUM") as psT:
        sums_ps = psA.tile([DB, C], F32)

        # Each 512-col half of sums_ps is one 2KB PSUM zero region: start=True
        # zeroes the WHOLE region, so exactly one start (global first MM into
        # that region) / one stop (global last); every other matmul
        # accumulates onto pending-zero bytes. Rows m != r get +0.
        def sum_slab(slab, r, first, last):
            st = ind_f[:, r, :].bitcast(F32R)
            njs = nj // 2
            for h in range(2):
                for j in range(njs):
                    nc.tensor.matmul(
                        sums_ps[:, h * 512:(h + 1) * 512],
                        st,
                        slab[:, j, h * 512:(h + 1) * 512],
                        start=(first and j == 0),
                        stop=(last and j == njs - 1),
                    )

        # one-time W_u transpose: wT[c, g] chunks (for the keys fixup)
        for k in range(NKC):
            tr = psT.tile([P, NKC * DB], F32, tag="fix")
            nc.tensor.transpose(tr[:, :G], w_sb[:, k * P:(k + 1) * P],
                                ident[:G, :G])
            nc.vector.tensor_copy(wT_sb[:, k, :], tr[:, :G])

        # half-slab streaming: 1MB tiles, 4 bufs — finer-grained buffer
        # recycling keeps DMA saturated despite the PE consumer lag
        nh = nj // 2
        cast_i = 0
        half_i = 0
        n_half = DB * 2
        for d in range(D):
            for b in range(B):
                for hh in range(2):
                    # tile is f32r (same bytes as f32) so the BIR verifier
                    # accepts it as a f32r-matmul moving operand
                    slab = stream.tile([P, nh, C], F32R, tag="slab")
                    # alternate halves across the two HWDGE rings
                    dmaeng = nc.sync if hh == 0 else nc.scalar
                    dmaeng.dma_start(
                        slab[:],
                        x[d, b, hh * nh * P:(hh + 1) * nh * P, :].rearrange(
                            "(j p) c -> p j c", p=P).bitcast(F32R))
                    if d in CACHED_D:
                        for j in range(nh):
                            dst = cache_bf[:, d, b, hh * nh + j, :]
                            src = slab[:, j, :].bitcast(F32)
                            if cast_i % 3 == 0:
                                nc.vector.tensor_copy(dst, src)
                            elif cast_i % 3 == 1:
                                nc.scalar.copy(dst, src)
                            else:
                                nc.gpsimd.tensor_copy(dst, src)
                            cast_i += 1
                    sum_slab(slab[:], d * B + b, first=(half_i == 0),
                             last=(half_i == n_half - 1))
                    half_i += 1

        nc.vector.tensor_copy(sums_sb[:], sums_ps[:])

        # ---- local partial keysT (keys are linear in the means, so the ----
        # ---- AllReduce can run in the tiny keys space: 8KB not 128KB)  ----
        # meanT[c, (d,b)] chunks via PE transpose — all 8 into one psum tile
        # (one zero region => single start/stop accumulation group)
        mt_ps = psT.tile([P, NKC * DB], F32, tag="fix")
        for k in range(NKC):
            nc.tensor.matmul(
                mt_ps[:, k * DB:(k + 1) * DB],
                sums_sb[:, k * P:(k + 1) * P], ident[:DB, :DB],
                is_transpose=True, start=(k == 0), stop=(k == NKC - 1))
        nc.vector.tensor_copy(meanT_sb[:], mt_ps[:])
        # partial keysT[g, (d b)] = sum_k wT_k.T @ meanT_k — one MM per k
        # over ALL (d,b); the (d b)->(b d) layout fix rides the DVE copy
        keys_ps = psT.tile([P, NKC * DB], F32, tag="fix")
        for k in range(NKC):
            nc.tensor.matmul(
                keys_ps[:G, :DB],
                wT_sb[:, k, :],
                meanT_sb[:, k * DB:(k + 1) * DB],
                start=(k == 0), stop=(k == NKC - 1),
            )
        nc.vector.tensor_copy(
            sumk_sb[:].rearrange("g (b d) -> g b d", b=B),
            keys_ps[:G, :DB].rearrange("g (d b) -> g d b", b=B).rearrange(
                "g d b -> g b d"))

    # extra phase-B prefetch pool in the (now free) stream-pool bytes:
    # its loads depend only on phase-A consumers, which finish before the
    # collective, so DMA keeps prefetching through the AllReduce latency
    bstream2 = es.enter_context(tc.tile_pool(name="bstream2", bufs=8))

    # ---------------- AllReduce the [G, B*D] partial keys -------------------
    # bounce DMAs go through GpSimd's queue so the Sync engine never blocks
    # on the collective and keeps issuing phase-B prefetch reads.
    nc.gpsimd.dma_start(cc_in[:], sumk_sb[:])
    nc.gpsimd.collective_compute(
        "AllReduce", add,
        replica_groups=[list(range(N_CORES))],
        ins=[cc_in.opt()], outs=[cc_out.opt()],
    )
    nc.gpsimd.dma_start(
        keysT_sb[:].rearrange("g b d -> g (b d)"), cc_out[:])

    # ---------------- Phase B: gates + depth-weighted sum -------------------
    with tc.tile_pool(name="psumX", bufs=2, space="PSUM") as psX, \
         tc.tile_pool(name="psumL", bufs=1, space="PSUM") as psL, \
         tc.tile_pool(name="psumW", bufs=2, space="PSUM") as psW, \
         tc.tile_pool(name="psumQ", bufs=1, space="PSUM") as psQ:
        # q[c, (b d)] = sum_g W_u[g, c] * keys[g, (b d)] — folds W_u into
        # keys so logits need x7^T only. 8 chunk matmuls into one psum tile
        # (single start/stop accumulation group), cast bf16 on ACT.
        q_ps = psQ.tile([P, NKC * B * D], F32)
        keys_mv = keysT_sb[:].rearrange("g b d -> g (b d)")
        for k in range(NKC):
            nc.tensor.matmul(
                q_ps[:, k * B * D:(k + 1) * B * D],
                w_sb[:, k * P:(k + 1) * P], keys_mv,
                start=(k == 0), stop=(k == NKC - 1))
        for k in range(NKC):
            nc.scalar.copy(
                q_bf[:, k, :, :],
                q_ps[:, k * B * D:(k + 1) * B * D].rearrange(
                    "p (b d) -> p b d", b=B))

        identr = ident_r[:]
        PE_D = (5, 6)           # streamed depths weighted on PE (f32r diag)
        DVE_D = (4,)            # streamed depths weighted on DVE FMAs
        for b in range(B):
            for j in range(nj):
                t = {}
                for d in STREAM_D:
                    dt_ = F32R if (d == 7 or d in PE_D) else F32
                    pool = bstream2 if d in (7, 4) else bstream
                    t[d] = pool.tile([P, C], dt_, tag="bslab",
                                     name=f"bs_{d}")
                    src = x[d, b, j * P:(j + 1) * P, :]
                    if dt_ is F32R:
                        src = src.bitcast(F32R)
                    # split the 5 loads across both HWDGE rings
                    dmaeng = nc.scalar if d in (5, 6) else nc.sync
                    dmaeng.dma_start(t[d][:], src)
                # x7^T chunks on PE (f32r transpose), bf16 stationaries
                xt_sb = xtp.tile([P, NKC, P], BF16, tag="xt")
                for k in range(NKC):
                    xt_ps = psX.tile([P, P], F32, tag="xtps")
                    nc.tensor.transpose(
                        xt_ps[:].bitcast(F32R),
                        t[7][:, k * P:(k + 1) * P],
                        identr)
                    nc.scalar.copy(xt_sb[:, k, :], xt_ps[:])
                lg_ps = psL.tile([P, D], F32, tag="lg")
                for k in range(NKC):
                    nc.tensor.matmul(
                        lg_ps[:], xt_sb[:, k, :], q_bf[:, k, b, :],
                        start=(k == 0), stop=(k == NKC - 1))
                e_sb = gsm.tile([P, D], F32, tag="e")
                z_sb = gsm.tile([P, 1], F32, tag="z")
                rz_sb = gsm.tile([P, 1], F32, tag="rz")
                gates = gsm.tile([P, D], F32, tag="gates")
                nc.scalar.activation(
                    e_sb[:], lg_ps[:], mybir.ActivationFunctionType.Exp,
                    accum_out=z_sb[:])
                nc.vector.reciprocal(rz_sb[:], z_sb[:])
                nc.scalar.mul(gates[:], e_sb[:], rz_sb[:])

                # PE path: diag(g_d) stationaries do per-row scaling; the
                # cached bf16 depths and two streamed f32r depths accumulate
                # in one PSUM region. diag builds: bf16 on ACT, f32r on DVE.
                dg_bf = dgp.tile([P, len(CACHED_D), P], BF16, tag="dgbf")
                for ci, d in enumerate(CACHED_D):
                    nc.scalar.mul(dg_bf[:, ci, :], ident[:],
                                  gates[:, d:d + 1])
                dg_r = dgp.tile([P, len(PE_D), P], F32R, tag="dgr")
                for ci, d in enumerate(PE_D):
                    nc.vector.tensor_scalar_mul(dg_r[:, ci, :], ident[:],
                                                gates[:, d:d + 1])
                # one MM output must stay within a single 2KB PSUM bank,
                # so every diag MM runs per 512-col half
                wsum_ps = psW.tile([P, C], F32, tag="wsum")
                for ci, d in enumerate(CACHED_D):
                    for h in range(2):
                        nc.tensor.matmul(
                            wsum_ps[:, h * 512:(h + 1) * 512],
                            dg_bf[:, ci, :],
                            cache_bf[:, d, b, j, h * 512:(h + 1) * 512],
                            start=(ci == 0), stop=False)
                for ci, d in enumerate(PE_D):
                    for h in range(2):
                        nc.tensor.matmul(
                            wsum_ps[:, h * 512:(h + 1) * 512],
                            dg_r[:, ci, :], t[d][:, h * 512:(h + 1) * 512],
                            start=False, stop=(ci == len(PE_D) - 1))

                # DVE path: t7 init + two streamed FMAs + PSUM combine
                acc = accp.tile([P, C], F32, tag="acc")
                nc.vector.tensor_scalar_mul(acc[:], t[7][:].bitcast(F32),
                                            gates[:, 7:8])
                for d in DVE_D:
                    nc.vector.scalar_tensor_tensor(
                        out=acc[:], in0=t[d][:], scalar=gates[:, d:d + 1],
                        in1=acc[:], op0=mul, op1=add)
                # final add emits bf16: halves the y write traffic (host
                # casts back to f32; well within the error budget)
                y_bf = ybfp.tile([P, C], BF16, tag="ybf")
                nc.vector.tensor_add(y_bf[:], acc[:], wsum_ps[:])
                # y writes via GpSimd SWDGE: both HWDGE rings stay loads-only
                nc.gpsimd.dma_start(y[b, j * P:(j + 1) * P, :], y_bf[:])

    es.close()


def build_nc(s_sh):
    nc = bacc.Bacc("TRN2", target_bir_lowering=False, debug=False,
                   num_devices=N_CORES)
    x_ap = nc.dram_tensor("x", [D, B, s_sh, C], F32, kind="ExternalInput").ap()
    w_ap = nc.dram_tensor("w", [G, C], F32, kind="ExternalInput").ap()
    y_ap = nc.dram_tensor("y", [B, s_sh, C], BF16,
                          kind="ExternalOutput").ap()
    with tile.TileContext(nc) as tc:
        build_body(tc, x_ap, w_ap, y_ap, s_sh)
    nc.compile()
    return nc


_NC_CACHE = {}


def _get_nc(s_sh):
    if s_sh not in _NC_CACHE:
        _NC_CACHE[s_sh] = build_nc(s_sh)
    return _NC_CACHE[s_sh]


def run(cached_states, W_u, trace=False, trace_cores=None):
    s_sh = S // N_CORES
    nc = _get_nc(s_sh)
    xs = np.asarray(cached_states, dtype=np.float32)
    ws = np.ascontiguousarray(np.asarray(W_u, dtype=np.float32))
    in_maps = []
    for i in range(N_CORES):
        sh = np.ascontiguousarray(xs[:, :, i * s_sh:(i + 1) * s_sh, :])
        in_maps.append({"x": sh, "w": ws})
    res = bass_utils.run_bass_kernel_spmd(
        nc, in_maps, core_ids=list(range(N_CORES)), trace=trace,
        trace_cores=trace_cores)
    out = np.empty((B, S, C), np.float32)
    for i in range(N_CORES):
        out[:, i * s_sh:(i + 1) * s_sh, :] = np.asarray(
            res.results[i]["y"]).astype(np.float32)
    return out, res


def kernel(cached_states, W_u):
    out, _ = run(cached_states, W_u)
    return out
